# revision 24
# baseline (speedup 1.0000x reference)
"""Trainium2 Bass kernel for nn_KLDivLossColBERTInBatch.

Math (see reference):
  q-hat = q / ||q||_D              (per query token, over feature dim)
  d-hat = (d*mask) / ||d*mask||_Ld (per (n,b,dfeat) column, over SEQUENCE dim!)
  S[n,bq,bd] = sum_l max_m <q-hat[bq,l], d-hat[n,bd,m]>   (MaxSim, all pairs)
  loss = KL(labels || softmax(S_diag)) + 0.5 * CE(in-batch)

Sharding (chosen; deviates from the hint on purpose): shard over the
N*B = 128 (n,bd) doc pairs, 16 per core, with queries replicated. This way
the doc-side normalization (the expensive elementwise part) is computed once
per pair instead of 8x redundantly, and there is no collective: each core
emits its [16, 64] score slab and the host concatenates.

Device layout: contraction dim D=128 lives on SBUF partitions for both
matmul operands. d is fed pre-transposed ([dfeat, pair, Ld] fp16; layout
prep on host), q is fed naturally (fp16) and transposed on the PE after
on-device normalization (norms accumulate in f32). Matmuls run fp16 ->
f32 PSUM in [128, 4 pairs, 256] tiles; the max over Ld is drained via big
ACT copies (cast fp16) + a DVE pairwise-max tree for 3 of every 4 tiles
and one direct DVE reduce_max for the 4th, balancing ACT and DVE. GPSIMD
computes the normalization squares; the Lq-sum is a PE matmul against a
0/1 group-indicator; the tiny softmax/KL/CE finale over the gathered
[128, 64] score slab runs on the host in float64.
"""

import numpy as np

try:
    import concourse.bass as bass
except ImportError:  # fresh grading dir: fall back to the container install
    import sys

    for p in ("/opt/trn_rl_repo", "/opt/pypackages"):
        if p not in sys.path:
            sys.path.append(p)
    import concourse.bass as bass

import concourse.bacc as bacc
import concourse.tile as tile
from concourse import mybir
from concourse.bass_utils import run_bass_kernel_spmd

F16 = mybir.dt.float16
F32 = mybir.dt.float32

N_CORES = 8
B, Lq, Ld, D, N = 64, 32, 256, 128, 2
PAIRS = N * B            # 128 (n, bd) doc pairs
P_LOC = PAIRS // N_CORES  # 16 pairs per core
QTOK = B * Lq            # 2048 query tokens (replicated on every core)
NQC = QTOK // 128        # 16 chunks of 128 tokens
INBATCH_P = 0.5

_PROG = None


def _build_program(n_dve=1, reps=1, qt_dve=False, s_act=True, simsb_bufs=3,
                   tree_qcs=1, dve_first=False, alt_dve=False, treep_bufs=2,
                   dma_split=False, dve_pos=3, split_tile=False):
    """Build the per-core SPMD Bass program.

    Tuned defaults (production cost model): per qchunk, 3 of 4 sim PSUM
    tiles drain via one big ACT copy (cast fp16) + DVE max-tree, 1 via a
    single direct DVE reduce_max; reps>1 repeats the whole body (timing).
    """
    nc = bacc.Bacc("TRN2", target_bir_lowering=False, debug=False, num_devices=N_CORES)
    dT_in = nc.dram_tensor("dt", [128, P_LOC, Ld], F16, kind="ExternalInput")
    q_in = nc.dram_tensor("q", [NQC, 128, D], F16, kind="ExternalInput")
    ind_in = nc.dram_tensor("ind", [128, 4], F16, kind="ExternalInput")
    ident_in = nc.dram_tensor("ident", [128, 128], F16, kind="ExternalInput")
    s_out = nc.dram_tensor("s", [P_LOC, B], F32, kind="ExternalOutput")

    AX = mybir.AxisListType.X
    MAX = mybir.AluOpType.max
    MULT = mybir.AluOpType.mult
    SQRT = mybir.ActivationFunctionType.Sqrt

    with tile.TileContext(nc) as tc:
        with tc.tile_pool(name="const", bufs=1) as cp:
            ind_t = cp.tile([128, 4], F16)
            nc.sync.dma_start(out=ind_t[:], in_=ind_in[:])
            ident = cp.tile([128, 128], F16)
            nc.sync.dma_start(out=ident[:], in_=ident_in[:])

            dT_f32 = cp.tile([128, P_LOC, Ld], F16)
            dsq = cp.tile([128, P_LOC, Ld], F32)     # GPS squares scratch
            q_f32 = cp.tile([128, NQC, D], F16)
            qsq = cp.tile([128, NQC, D], F32)
            qT = cp.tile([128, QTOK], F16)           # [dfeat, qtok] normalized
            dT_s = cp.tile([128, P_LOC, Ld], F16)    # [dfeat, pair, m] normalized
            maxtile = cp.tile([128, NQC, P_LOC], F16)
            dn2 = cp.tile([128, P_LOC], F32)
            dn = cp.tile([128, P_LOC], F32)
            ds = cp.tile([128, P_LOC], F32)
            qn2 = cp.tile([128, NQC], F32)
            qn = cp.tile([128, NQC], F32)
            qs = cp.tile([128, NQC], F32)
            S_sbuf = cp.tile([P_LOC, B], F32)
            eps_t = cp.tile([128, 1], F32)
            nc.vector.memset(eps_t, 1e-24)

            for _rep in range(reps):
                # ---- d prep (pipelined in groups of 4 pairs) ----
                # norm over Ld (free axis) per (pair, dfeat): GPS squares,
                # DVE reduce, ACT sqrt(+eps), DVE recip + scale-cast.
                for g in range(0, P_LOC, 4):
                    nc.sync.dma_start(
                        out=dT_f32[:, g:g + 4, :], in_=dT_in[:, g:g + 4, :])
                    nc.gpsimd.tensor_mul(
                        dsq[:, g:g + 4, :], dT_f32[:, g:g + 4, :],
                        dT_f32[:, g:g + 4, :])
                    nc.vector.reduce_sum(
                        dn2[:, g:g + 4], dsq[:, g:g + 4, :], axis=AX)
                    nc.scalar.activation(
                        dn[:, g:g + 4], dn2[:, g:g + 4], SQRT, bias=eps_t[:])
                    nc.vector.reciprocal(ds[:, g:g + 4], dn[:, g:g + 4])
                    for p in range(g, g + 4):
                        nc.vector.tensor_scalar_mul(
                            dT_s[:, p, :], dT_f32[:, p, :], ds[:, p:p + 1])

                # ---- q prep (pipelined in groups of 4 chunks) ----
                simps_cm = tc.tile_pool(name="simps", bufs=3, space="PSUM")
                simps = simps_cm.__enter__()
                with tc.tile_pool(name="qprep", bufs=3) as qp:
                    for g in range(0, NQC, 4):
                        for qc in range(g, g + 4):
                            nc.sync.dma_start(
                                out=q_f32[:, qc, :], in_=q_in[qc])
                        nc.gpsimd.tensor_mul(
                            qsq[:, g:g + 4, :], q_f32[:, g:g + 4, :],
                            q_f32[:, g:g + 4, :])
                        nc.vector.reduce_sum(
                            qn2[:, g:g + 4], qsq[:, g:g + 4, :], axis=AX)
                        nc.scalar.activation(
                            qn[:, g:g + 4], qn2[:, g:g + 4], SQRT, bias=eps_t[:])
                        nc.vector.reciprocal(qs[:, g:g + 4], qn[:, g:g + 4])
                        for qc in range(g, g + 4):
                            qn16 = qp.tile([128, D], F16, tag="qn16")
                            nc.vector.tensor_scalar_mul(
                                qn16[:], q_f32[:, qc, :], qs[:, qc:qc + 1])
                            qT_ps = simps.tile([128, 128], F16, tag="sim")
                            nc.tensor.transpose(qT_ps[:], qn16[:], ident[:])
                            nc.scalar.copy(qT[:, qc * 128:(qc + 1) * 128], qT_ps[:])

                # ---- main: sim matmuls (f32 PSUM) + max over Ld ----
                # Per qc: 4 PSUM tiles of 4 pairs (2 banks each, bufs=3).
                # 3 tiles are ACT-copied (cast fp16) into simc and max-reduced by
                # a DVE tree; 1 tile drains via one direct DVE reduce_max.
                n_act = 4 - n_dve
                with tc.tile_pool(name="simsb", bufs=simsb_bufs) as simsb, \
                     tc.tile_pool(name="tree", bufs=treep_bufs) as treep:
                    for qcg in range(0, NQC, tree_qcs):
                        simc = simsb.tile(
                            [128, tree_qcs * 4 * n_act, Ld], F16, tag="simc")
                        for qi in range(tree_qcs):
                            qc = qcg + qi
                            nd = n_dve + (1 if (alt_dve and qc % 2) else 0)
                            na = 4 - nd
                            lhs = qT[:, qc * 128:(qc + 1) * 128]
                            order = [t for t in range(4) if t != dve_pos]
                            order = ([dve_pos] + order if dve_first
                                     else order[:dve_pos] + [dve_pos]
                                     + order[dve_pos:])
                            assert not (dve_first and alt_dve)
                            for t in order:
                                sim = simps.tile([128, 4, Ld], F32, tag="sim")
                                for b in range(2):
                                    pr = 4 * t + 2 * b
                                    nc.tensor.matmul(
                                        sim[:, 2 * b:2 * b + 2, :], lhs,
                                        dT_s[:, pr:pr + 2, :], start=True,
                                        stop=True)
                                if split_tile and t == na - 1:
                                    # 3 pairs via ACT copy, 1 via direct DVE
                                    nc.scalar.copy(
                                        simc[:, 4 * t:4 * t + 3, :],
                                        sim[:, 0:3, :])
                                    nc.vector.reduce_max(
                                        maxtile[:, qc, 4 * t + 3:4 * t + 4],
                                        sim[:, 3:4, :], axis=AX)
                                elif t < na:
                                    nc.scalar.copy(
                                        simc[:, qi * 4 * n_act + 4 * t:
                                             qi * 4 * n_act + 4 * t + 4, :],
                                        sim[:])
                                else:
                                    nc.vector.reduce_max(
                                        maxtile[:, qc, 4 * t:4 * t + 4],
                                        sim[:], axis=AX)
                        np_ = 4 * na if alt_dve else tree_qcs * 4 * n_act
                        if split_tile:
                            np_ = 4 * n_act - 1
                        t1 = treep.tile([128, tree_qcs * 4 * n_act, 128], F16,
                                        tag="t1")
                        nc.vector.tensor_tensor(
                            t1[:, 0:np_, :], simc[:, 0:np_, 0:128],
                            simc[:, 0:np_, 128:256], op=MAX)
                        t2 = treep.tile([128, tree_qcs * 4 * n_act, 64], F16,
                                        tag="t2")
                        nc.vector.tensor_tensor(
                            t2[:, 0:np_, :], t1[:, 0:np_, 0:64],
                            t1[:, 0:np_, 64:128], op=MAX)
                        t3 = treep.tile([128, tree_qcs * 4 * n_act, 32], F16,
                                        tag="t3")
                        nc.vector.tensor_tensor(
                            t3[:, 0:np_, :], t2[:, 0:np_, 0:32],
                            t2[:, 0:np_, 32:64], op=MAX)
                        t4 = treep.tile([128, tree_qcs * 4 * n_act, 16], F16,
                                        tag="t4")
                        nc.vector.tensor_tensor(
                            t4[:, 0:np_, :], t3[:, 0:np_, 0:16],
                            t3[:, 0:np_, 16:32], op=MAX)
                        t5 = treep.tile([128, tree_qcs * 4 * n_act, 8], F16,
                                        tag="t5")
                        nc.vector.tensor_tensor(
                            t5[:, 0:np_, :], t4[:, 0:np_, 0:8],
                            t4[:, 0:np_, 8:16], op=MAX)
                        for qi in range(tree_qcs):
                            qc = qcg + qi
                            w = np_ if (alt_dve or split_tile) else 4 * n_act
                            nc.vector.reduce_max(
                                maxtile[:, qc, 0:w],
                                t5[:, qi * 4 * n_act:qi * 4 * n_act + w, :],
                                axis=AX)
                            s_ps = simps.tile(
                                [P_LOC, 4], F32, tag="sprs", bufs=2)
                            nc.tensor.matmul(
                                s_ps[:], maxtile[:, qc, :], ind_t[:],
                                start=True, stop=True)
                            if s_act:
                                nc.scalar.copy(
                                    S_sbuf[:, 4 * qc:4 * qc + 4], s_ps[:])
                            else:
                                nc.vector.tensor_copy(
                                    S_sbuf[:, 4 * qc:4 * qc + 4], s_ps[:])
                simps_cm.__exit__(None, None, None)

                if dma_split:
                    for g4 in range(0, NQC, 4):
                        nc.sync.dma_start(
                            out=s_out[:, 4 * g4:4 * g4 + 16],
                            in_=S_sbuf[:, 4 * g4:4 * g4 + 16])
                else:
                    nc.sync.dma_start(out=s_out[:], in_=S_sbuf[:])

    nc.finalize()
    return nc


def _get_program():
    global _PROG
    if _PROG is None:
        _PROG = _build_program()
    return _PROG


def _logsumexp(x, axis):
    m = np.max(x, axis=axis, keepdims=True)
    return (m + np.log(np.sum(np.exp(x - m), axis=axis, keepdims=True)))


def kernel(q_reps, d_reps, d_masks, labels):
    nc = _get_program()
    q_reps = np.asarray(q_reps, dtype=np.float32)
    d_reps = np.asarray(d_reps, dtype=np.float32)
    d_masks = np.asarray(d_masks)
    labels = np.asarray(labels, dtype=np.float32)

    # Host-side input layout prep (mask, shard, transpose so that the
    # contraction dim D sits on SBUF partitions).
    d = d_reps * d_masks[..., None].astype(d_reps.dtype)          # [N,B,Ld,D]
    dT = np.transpose(d.reshape(PAIRS, Ld, D), (2, 0, 1))          # [D,pair,Ld]
    q = np.ascontiguousarray(
        q_reps.reshape(QTOK, D).reshape(NQC, 128, D), dtype=np.float32)
    ind = np.zeros((128, 4), np.float16)
    ind[np.arange(128), np.arange(128) // 32] = 1
    ident = np.eye(128, dtype=np.float16)

    in_maps = []
    for c in range(N_CORES):
        in_maps.append({
            "dt": np.ascontiguousarray(
                dT[:, c * P_LOC:(c + 1) * P_LOC, :]).astype(np.float16),
            "q": q.astype(np.float16),
            "ind": ind,
            "ident": ident,
        })
    res = run_bass_kernel_spmd(nc, in_maps, list(range(N_CORES)))

    # Gather the per-core [16, 64] score slabs -> S[n, bq, bd].
    S_full = np.concatenate(
        [res.results[c]["s"] for c in range(N_CORES)], axis=0)     # [128, 64]
    S = S_full.reshape(N, B, B).transpose(0, 2, 1).astype(np.float64)

    # Tiny scalar finale (softmax losses over the gathered 128x64 slab).
    lab = labels.astype(np.float64)
    idx = np.arange(B)
    scores = S[:, idx, idx].T                                      # [B, N]
    log_p = scores - _logsumexp(scores, axis=-1)
    kl = np.sum(np.exp(lab) * (lab - log_p)) / B
    pred = S.transpose(1, 0, 2).reshape(B, N * B)                  # [B, N*B]
    ce = -np.mean(pred[idx, idx] - _logsumexp(pred, axis=-1)[:, 0])
    return np.float32(kl + INBATCH_P * ce)



# revision 26
# speedup vs baseline: 1.0127x; 1.0127x over previous
"""Trainium2 Bass kernel for nn_KLDivLossColBERTInBatch.

Math (see reference):
  q-hat = q / ||q||_D              (per query token, over feature dim)
  d-hat = (d*mask) / ||d*mask||_Ld (per (n,b,dfeat) column, over SEQUENCE dim!)
  S[n,bq,bd] = sum_l max_m <q-hat[bq,l], d-hat[n,bd,m]>   (MaxSim, all pairs)
  loss = KL(labels || softmax(S_diag)) + 0.5 * CE(in-batch)

Sharding (chosen; deviates from the hint on purpose): shard over the
N*B = 128 (n,bd) doc pairs, 16 per core, with queries replicated. This way
the doc-side normalization (the expensive elementwise part) is computed once
per pair instead of 8x redundantly, and there is no collective: each core
emits its [16, 64] score slab and the host concatenates.

Device layout: contraction dim D=128 lives on SBUF partitions for both
matmul operands. d is fed pre-transposed ([dfeat, pair, Ld] fp16; layout
prep on host), q is fed naturally (fp16) and transposed on the PE after
on-device normalization (norms accumulate in f32). Matmuls run fp16 ->
f32 PSUM in [128, 4 pairs, 256] tiles; the max over Ld is drained via big
ACT copies (cast fp16) + a DVE pairwise-max tree for 3 of every 4 tiles
and one direct DVE reduce_max for the 4th, balancing ACT and DVE. GPSIMD
computes the normalization squares; the Lq-sum is a PE matmul against a
0/1 group-indicator; the tiny softmax/KL/CE finale over the gathered
[128, 64] score slab runs on the host in float64.
"""

import numpy as np

try:
    import concourse.bass as bass
except ImportError:  # fresh grading dir: fall back to the container install
    import sys

    for p in ("/opt/trn_rl_repo", "/opt/pypackages"):
        if p not in sys.path:
            sys.path.append(p)
    import concourse.bass as bass

import concourse.bacc as bacc
import concourse.tile as tile
from concourse import mybir
from concourse.bass_utils import run_bass_kernel_spmd

F16 = mybir.dt.float16
F32 = mybir.dt.float32

N_CORES = 8
B, Lq, Ld, D, N = 64, 32, 256, 128, 2
PAIRS = N * B            # 128 (n, bd) doc pairs
P_LOC = PAIRS // N_CORES  # 16 pairs per core
QTOK = B * Lq            # 2048 query tokens (replicated on every core)
NQC = QTOK // 128        # 16 chunks of 128 tokens
INBATCH_P = 0.5

_PROG = None


def _build_program(n_dve=1, reps=1, qt_dve=False, s_act=True, simsb_bufs=3,
                   tree_qcs=1, dve_first=False, alt_dve=False, treep_bufs=2,
                   dma_split=False, dve_pos=3, split_tile=False,
                   dve_g0=True):
    """Build the per-core SPMD Bass program.

    Tuned defaults (production cost model): per qchunk, 3 of 4 sim PSUM
    tiles drain via one big ACT copy (cast fp16) + DVE max-tree, 1 via a
    single direct DVE reduce_max; reps>1 repeats the whole body (timing).
    """
    nc = bacc.Bacc("TRN2", target_bir_lowering=False, debug=False, num_devices=N_CORES)
    dT_in = nc.dram_tensor("dt", [128, P_LOC, Ld], F16, kind="ExternalInput")
    q_in = nc.dram_tensor("q", [NQC, 128, D], F16, kind="ExternalInput")
    ind_in = nc.dram_tensor("ind", [128, 4], F16, kind="ExternalInput")
    ident_in = nc.dram_tensor("ident", [128, 128], F16, kind="ExternalInput")
    s_out = nc.dram_tensor("s", [P_LOC, B], F32, kind="ExternalOutput")

    AX = mybir.AxisListType.X
    MAX = mybir.AluOpType.max
    MULT = mybir.AluOpType.mult
    SQRT = mybir.ActivationFunctionType.Sqrt

    with tile.TileContext(nc) as tc:
        with tc.tile_pool(name="const", bufs=1) as cp:
            ind_t = cp.tile([128, 4], F16)
            nc.sync.dma_start(out=ind_t[:], in_=ind_in[:])
            ident = cp.tile([128, 128], F16)
            nc.sync.dma_start(out=ident[:], in_=ident_in[:])

            dT_f32 = cp.tile([128, P_LOC, Ld], F16)
            dsq = cp.tile([128, P_LOC, Ld], F32)     # GPS squares scratch
            q_f32 = cp.tile([128, NQC, D], F16)
            qsq = cp.tile([128, NQC, D], F32)
            qT = cp.tile([128, QTOK], F16)           # [dfeat, qtok] normalized
            dT_s = cp.tile([128, P_LOC, Ld], F16)    # [dfeat, pair, m] normalized
            maxtile = cp.tile([128, NQC, P_LOC], F16)
            dn2 = cp.tile([128, P_LOC], F32)
            dn = cp.tile([128, P_LOC], F32)
            ds = cp.tile([128, P_LOC], F32)
            qn2 = cp.tile([128, NQC], F32)
            qn = cp.tile([128, NQC], F32)
            qs = cp.tile([128, NQC], F32)
            S_sbuf = cp.tile([P_LOC, B], F32)
            eps_t = cp.tile([128, 1], F32)
            nc.vector.memset(eps_t, 1e-24)

            for _rep in range(reps):
                # ---- d prep (pipelined in groups of 4 pairs) ----
                # norm over Ld (free axis) per (pair, dfeat): GPS squares,
                # DVE reduce, ACT sqrt(+eps), DVE recip + scale-cast.
                for g in range(0, P_LOC, 4):
                    nc.sync.dma_start(
                        out=dT_f32[:, g:g + 4, :], in_=dT_in[:, g:g + 4, :])
                    sq_eng = nc.vector if (dve_g0 and g == 0) else nc.gpsimd
                    sq_eng.tensor_mul(
                        dsq[:, g:g + 4, :], dT_f32[:, g:g + 4, :],
                        dT_f32[:, g:g + 4, :])
                    nc.vector.reduce_sum(
                        dn2[:, g:g + 4], dsq[:, g:g + 4, :], axis=AX)
                    nc.scalar.activation(
                        dn[:, g:g + 4], dn2[:, g:g + 4], SQRT, bias=eps_t[:])
                    nc.vector.reciprocal(ds[:, g:g + 4], dn[:, g:g + 4])
                    for p in range(g, g + 4):
                        nc.vector.tensor_scalar_mul(
                            dT_s[:, p, :], dT_f32[:, p, :], ds[:, p:p + 1])

                # ---- q prep (pipelined in groups of 4 chunks) ----
                simps_cm = tc.tile_pool(name="simps", bufs=3, space="PSUM")
                simps = simps_cm.__enter__()
                with tc.tile_pool(name="qprep", bufs=3) as qp:
                    for g in range(0, NQC, 4):
                        for qc in range(g, g + 4):
                            nc.sync.dma_start(
                                out=q_f32[:, qc, :], in_=q_in[qc])
                        qsq_eng = (nc.vector if (dve_g0 and g == 0)
                                   else nc.gpsimd)
                        qsq_eng.tensor_mul(
                            qsq[:, g:g + 4, :], q_f32[:, g:g + 4, :],
                            q_f32[:, g:g + 4, :])
                        nc.vector.reduce_sum(
                            qn2[:, g:g + 4], qsq[:, g:g + 4, :], axis=AX)
                        nc.scalar.activation(
                            qn[:, g:g + 4], qn2[:, g:g + 4], SQRT, bias=eps_t[:])
                        nc.vector.reciprocal(qs[:, g:g + 4], qn[:, g:g + 4])
                        for qc in range(g, g + 4):
                            qn16 = qp.tile([128, D], F16, tag="qn16")
                            nc.vector.tensor_scalar_mul(
                                qn16[:], q_f32[:, qc, :], qs[:, qc:qc + 1])
                            qT_ps = simps.tile([128, 128], F16, tag="sim")
                            nc.tensor.transpose(qT_ps[:], qn16[:], ident[:])
                            nc.scalar.copy(qT[:, qc * 128:(qc + 1) * 128], qT_ps[:])

                # ---- main: sim matmuls (f32 PSUM) + max over Ld ----
                # Per qc: 4 PSUM tiles of 4 pairs (2 banks each, bufs=3).
                # 3 tiles are ACT-copied (cast fp16) into simc and max-reduced by
                # a DVE tree; 1 tile drains via one direct DVE reduce_max.
                n_act = 4 - n_dve
                with tc.tile_pool(name="simsb", bufs=simsb_bufs) as simsb, \
                     tc.tile_pool(name="tree", bufs=treep_bufs) as treep:
                    for qcg in range(0, NQC, tree_qcs):
                        simc = simsb.tile(
                            [128, tree_qcs * 4 * n_act, Ld], F16, tag="simc")
                        for qi in range(tree_qcs):
                            qc = qcg + qi
                            nd = n_dve + (1 if (alt_dve and qc % 2) else 0)
                            na = 4 - nd
                            lhs = qT[:, qc * 128:(qc + 1) * 128]
                            order = [t for t in range(4) if t != dve_pos]
                            order = ([dve_pos] + order if dve_first
                                     else order[:dve_pos] + [dve_pos]
                                     + order[dve_pos:])
                            assert not (dve_first and alt_dve)
                            for t in order:
                                sim = simps.tile([128, 4, Ld], F32, tag="sim")
                                for b in range(2):
                                    pr = 4 * t + 2 * b
                                    nc.tensor.matmul(
                                        sim[:, 2 * b:2 * b + 2, :], lhs,
                                        dT_s[:, pr:pr + 2, :], start=True,
                                        stop=True)
                                if split_tile and t == na - 1:
                                    # 3 pairs via ACT copy, 1 via direct DVE
                                    nc.scalar.copy(
                                        simc[:, 4 * t:4 * t + 3, :],
                                        sim[:, 0:3, :])
                                    nc.vector.reduce_max(
                                        maxtile[:, qc, 4 * t + 3:4 * t + 4],
                                        sim[:, 3:4, :], axis=AX)
                                elif t < na:
                                    nc.scalar.copy(
                                        simc[:, qi * 4 * n_act + 4 * t:
                                             qi * 4 * n_act + 4 * t + 4, :],
                                        sim[:])
                                else:
                                    nc.vector.reduce_max(
                                        maxtile[:, qc, 4 * t:4 * t + 4],
                                        sim[:], axis=AX)
                        np_ = 4 * na if alt_dve else tree_qcs * 4 * n_act
                        if split_tile:
                            np_ = 4 * n_act - 1
                        t1 = treep.tile([128, tree_qcs * 4 * n_act, 128], F16,
                                        tag="t1")
                        nc.vector.tensor_tensor(
                            t1[:, 0:np_, :], simc[:, 0:np_, 0:128],
                            simc[:, 0:np_, 128:256], op=MAX)
                        t2 = treep.tile([128, tree_qcs * 4 * n_act, 64], F16,
                                        tag="t2")
                        nc.vector.tensor_tensor(
                            t2[:, 0:np_, :], t1[:, 0:np_, 0:64],
                            t1[:, 0:np_, 64:128], op=MAX)
                        t3 = treep.tile([128, tree_qcs * 4 * n_act, 32], F16,
                                        tag="t3")
                        nc.vector.tensor_tensor(
                            t3[:, 0:np_, :], t2[:, 0:np_, 0:32],
                            t2[:, 0:np_, 32:64], op=MAX)
                        t4 = treep.tile([128, tree_qcs * 4 * n_act, 16], F16,
                                        tag="t4")
                        nc.vector.tensor_tensor(
                            t4[:, 0:np_, :], t3[:, 0:np_, 0:16],
                            t3[:, 0:np_, 16:32], op=MAX)
                        t5 = treep.tile([128, tree_qcs * 4 * n_act, 8], F16,
                                        tag="t5")
                        nc.vector.tensor_tensor(
                            t5[:, 0:np_, :], t4[:, 0:np_, 0:8],
                            t4[:, 0:np_, 8:16], op=MAX)
                        for qi in range(tree_qcs):
                            qc = qcg + qi
                            w = np_ if (alt_dve or split_tile) else 4 * n_act
                            nc.vector.reduce_max(
                                maxtile[:, qc, 0:w],
                                t5[:, qi * 4 * n_act:qi * 4 * n_act + w, :],
                                axis=AX)
                            s_ps = simps.tile(
                                [P_LOC, 4], F32, tag="sprs", bufs=2)
                            nc.tensor.matmul(
                                s_ps[:], maxtile[:, qc, :], ind_t[:],
                                start=True, stop=True)
                            if s_act:
                                nc.scalar.copy(
                                    S_sbuf[:, 4 * qc:4 * qc + 4], s_ps[:])
                            else:
                                nc.vector.tensor_copy(
                                    S_sbuf[:, 4 * qc:4 * qc + 4], s_ps[:])
                simps_cm.__exit__(None, None, None)

                if dma_split:
                    for g4 in range(0, NQC, 4):
                        nc.sync.dma_start(
                            out=s_out[:, 4 * g4:4 * g4 + 16],
                            in_=S_sbuf[:, 4 * g4:4 * g4 + 16])
                else:
                    nc.sync.dma_start(out=s_out[:], in_=S_sbuf[:])

    nc.finalize()
    return nc


def _get_program():
    global _PROG
    if _PROG is None:
        _PROG = _build_program()
    return _PROG


def _logsumexp(x, axis):
    m = np.max(x, axis=axis, keepdims=True)
    return (m + np.log(np.sum(np.exp(x - m), axis=axis, keepdims=True)))


def kernel(q_reps, d_reps, d_masks, labels):
    nc = _get_program()
    q_reps = np.asarray(q_reps, dtype=np.float32)
    d_reps = np.asarray(d_reps, dtype=np.float32)
    d_masks = np.asarray(d_masks)
    labels = np.asarray(labels, dtype=np.float32)

    # Host-side input layout prep (mask, shard, transpose so that the
    # contraction dim D sits on SBUF partitions).
    d = d_reps * d_masks[..., None].astype(d_reps.dtype)          # [N,B,Ld,D]
    dT = np.transpose(d.reshape(PAIRS, Ld, D), (2, 0, 1))          # [D,pair,Ld]
    q = np.ascontiguousarray(
        q_reps.reshape(QTOK, D).reshape(NQC, 128, D), dtype=np.float32)
    ind = np.zeros((128, 4), np.float16)
    ind[np.arange(128), np.arange(128) // 32] = 1
    ident = np.eye(128, dtype=np.float16)

    in_maps = []
    for c in range(N_CORES):
        in_maps.append({
            "dt": np.ascontiguousarray(
                dT[:, c * P_LOC:(c + 1) * P_LOC, :]).astype(np.float16),
            "q": q.astype(np.float16),
            "ind": ind,
            "ident": ident,
        })
    res = run_bass_kernel_spmd(nc, in_maps, list(range(N_CORES)))

    # Gather the per-core [16, 64] score slabs -> S[n, bq, bd].
    S_full = np.concatenate(
        [res.results[c]["s"] for c in range(N_CORES)], axis=0)     # [128, 64]
    S = S_full.reshape(N, B, B).transpose(0, 2, 1).astype(np.float64)

    # Tiny scalar finale (softmax losses over the gathered 128x64 slab).
    lab = labels.astype(np.float64)
    idx = np.arange(B)
    scores = S[:, idx, idx].T                                      # [B, N]
    log_p = scores - _logsumexp(scores, axis=-1)
    kl = np.sum(np.exp(lab) * (lab - log_p)) / B
    pred = S.transpose(1, 0, 2).reshape(B, N * B)                  # [B, N*B]
    ce = -np.mean(pred[idx, idx] - _logsumexp(pred, axis=-1)[:, 0])
    return np.float32(kl + INBATCH_P * ce)



# revision 29
# speedup vs baseline: 1.0188x; 1.0060x over previous
"""Trainium2 Bass kernel for nn_KLDivLossColBERTInBatch.

Math (see reference):
  q-hat = q / ||q||_D              (per query token, over feature dim)
  d-hat = (d*mask) / ||d*mask||_Ld (per (n,b,dfeat) column, over SEQUENCE dim!)
  S[n,bq,bd] = sum_l max_m <q-hat[bq,l], d-hat[n,bd,m]>   (MaxSim, all pairs)
  loss = KL(labels || softmax(S_diag)) + 0.5 * CE(in-batch)

Sharding (chosen; deviates from the hint on purpose): shard over the
N*B = 128 (n,bd) doc pairs, 16 per core, with queries replicated. This way
the doc-side normalization (the expensive elementwise part) is computed once
per pair instead of 8x redundantly, and there is no collective: each core
emits its [16, 64] score slab and the host concatenates.

Device layout: contraction dim D=128 lives on SBUF partitions for both
matmul operands. d is fed pre-transposed ([dfeat, pair, Ld] fp16; layout
prep on host), q is fed naturally (fp16) and transposed on the PE after
on-device normalization (norms accumulate in f32). Matmuls run fp16 ->
f32 PSUM in [128, 4 pairs, 256] tiles; the max over Ld is drained via big
ACT copies (cast fp16) + a DVE pairwise-max tree for 3 of every 4 tiles
and one direct DVE reduce_max for the 4th, balancing ACT and DVE. GPSIMD
computes the normalization squares; the Lq-sum is a PE matmul against a
0/1 group-indicator; the tiny softmax/KL/CE finale over the gathered
[128, 64] score slab runs on the host in float64.
"""

import numpy as np

try:
    import concourse.bass as bass
except ImportError:  # fresh grading dir: fall back to the container install
    import sys

    for p in ("/opt/trn_rl_repo", "/opt/pypackages"):
        if p not in sys.path:
            sys.path.append(p)
    import concourse.bass as bass

import concourse.bacc as bacc
import concourse.tile as tile
from concourse import mybir
from concourse.bass_utils import run_bass_kernel_spmd

F16 = mybir.dt.float16
F32 = mybir.dt.float32

N_CORES = 8
B, Lq, Ld, D, N = 64, 32, 256, 128, 2
PAIRS = N * B            # 128 (n, bd) doc pairs
P_LOC = PAIRS // N_CORES  # 16 pairs per core
QTOK = B * Lq            # 2048 query tokens (replicated on every core)
NQC = QTOK // 128        # 16 chunks of 128 tokens
INBATCH_P = 0.5

_PROG = None


def _build_program(n_dve=1, reps=1, qt_dve=False, s_act=True, simsb_bufs=3,
                   tree_qcs=1, dve_first=False, alt_dve=False, treep_bufs=2,
                   dma_split=False, dve_pos=3, split_tile=False,
                   dve_g0=True, dve_g01=True, act_norm=False):
    """Build the per-core SPMD Bass program.

    Tuned defaults (production cost model): per qchunk, 3 of 4 sim PSUM
    tiles drain via one big ACT copy (cast fp16) + DVE max-tree, 1 via a
    single direct DVE reduce_max; reps>1 repeats the whole body (timing).
    """
    nc = bacc.Bacc("TRN2", target_bir_lowering=False, debug=False, num_devices=N_CORES)
    dT_in = nc.dram_tensor("dt", [128, P_LOC, Ld], F16, kind="ExternalInput")
    q_in = nc.dram_tensor("q", [NQC, 128, D], F16, kind="ExternalInput")
    ind_in = nc.dram_tensor("ind", [128, 4], F16, kind="ExternalInput")
    ident_in = nc.dram_tensor("ident", [128, 128], F16, kind="ExternalInput")
    s_out = nc.dram_tensor("s", [P_LOC, B], F32, kind="ExternalOutput")

    AX = mybir.AxisListType.X
    MAX = mybir.AluOpType.max
    MULT = mybir.AluOpType.mult
    SQRT = mybir.ActivationFunctionType.Sqrt

    with tile.TileContext(nc) as tc:
        with tc.tile_pool(name="const", bufs=1) as cp:
            ind_t = cp.tile([128, 4], F16)
            nc.sync.dma_start(out=ind_t[:], in_=ind_in[:])
            ident = cp.tile([128, 128], F16)
            nc.sync.dma_start(out=ident[:], in_=ident_in[:])

            dT_f32 = cp.tile([128, P_LOC, Ld], F16)
            dsq = cp.tile([128, P_LOC, Ld], F32)     # GPS squares scratch
            q_f32 = cp.tile([128, NQC, D], F16)
            qsq = cp.tile([128, NQC, D], F32)
            qT = cp.tile([128, QTOK], F16)           # [dfeat, qtok] normalized
            dT_s = cp.tile([128, P_LOC, Ld], F16)    # [dfeat, pair, m] normalized
            maxtile = cp.tile([128, NQC, P_LOC], F16)
            dn2 = cp.tile([128, P_LOC], F32)
            dn = cp.tile([128, P_LOC], F32)
            ds = cp.tile([128, P_LOC], F32)
            qn2 = cp.tile([128, NQC], F32)
            qn = cp.tile([128, NQC], F32)
            qs = cp.tile([128, NQC], F32)
            S_sbuf = cp.tile([P_LOC, B], F32)
            eps_t = cp.tile([128, 1], F32)
            nc.vector.memset(eps_t, 1e-24)

            for _rep in range(reps):
                # ---- d prep (pipelined in groups of 4 pairs) ----
                # norm over Ld (free axis) per (pair, dfeat): GPS squares,
                # DVE reduce, ACT sqrt(+eps), DVE recip + scale-cast.
                for g in range(0, P_LOC, 4):
                    nc.sync.dma_start(
                        out=dT_f32[:, g:g + 4, :], in_=dT_in[:, g:g + 4, :])
                    early = g == 0 or (dve_g01 and g == 4) or act_norm
                    sq_eng = nc.vector if (dve_g0 and early) else nc.gpsimd
                    sq_eng.tensor_mul(
                        dsq[:, g:g + 4, :], dT_f32[:, g:g + 4, :],
                        dT_f32[:, g:g + 4, :])
                    nc.vector.reduce_sum(
                        dn2[:, g:g + 4], dsq[:, g:g + 4, :], axis=AX)
                    nc.scalar.activation(
                        dn[:, g:g + 4], dn2[:, g:g + 4], SQRT, bias=eps_t[:])
                    nc.vector.reciprocal(ds[:, g:g + 4], dn[:, g:g + 4])
                    for p in range(g, g + 4):
                        nc.vector.tensor_scalar_mul(
                            dT_s[:, p, :], dT_f32[:, p, :], ds[:, p:p + 1])

                # ---- q prep (pipelined in groups of 4 chunks) ----
                simps_cm = tc.tile_pool(name="simps", bufs=3, space="PSUM")
                simps = simps_cm.__enter__()
                with tc.tile_pool(name="qprep", bufs=3) as qp:
                    for g in range(0, NQC, 4):
                        for qc in range(g, g + 4):
                            nc.sync.dma_start(
                                out=q_f32[:, qc, :], in_=q_in[qc])
                        qearly = g == 0 or (dve_g01 and g == 4) or act_norm
                        qsq_eng = (nc.vector if (dve_g0 and qearly)
                                   else nc.gpsimd)
                        qsq_eng.tensor_mul(
                            qsq[:, g:g + 4, :], q_f32[:, g:g + 4, :],
                            q_f32[:, g:g + 4, :])
                        nc.vector.reduce_sum(
                            qn2[:, g:g + 4], qsq[:, g:g + 4, :], axis=AX)
                        nc.scalar.activation(
                            qn[:, g:g + 4], qn2[:, g:g + 4], SQRT, bias=eps_t[:])
                        nc.vector.reciprocal(qs[:, g:g + 4], qn[:, g:g + 4])
                        for qc in range(g, g + 4):
                            qn16 = qp.tile([128, D], F16, tag="qn16")
                            nc.vector.tensor_scalar_mul(
                                qn16[:], q_f32[:, qc, :], qs[:, qc:qc + 1])
                            qT_ps = simps.tile([128, 128], F16, tag="sim")
                            nc.tensor.transpose(qT_ps[:], qn16[:], ident[:])
                            nc.scalar.copy(qT[:, qc * 128:(qc + 1) * 128], qT_ps[:])

                # ---- main: sim matmuls (f32 PSUM) + max over Ld ----
                # Per qc: 4 PSUM tiles of 4 pairs (2 banks each, bufs=3).
                # 3 tiles are ACT-copied (cast fp16) into simc and max-reduced by
                # a DVE tree; 1 tile drains via one direct DVE reduce_max.
                n_act = 4 - n_dve
                with tc.tile_pool(name="simsb", bufs=simsb_bufs) as simsb, \
                     tc.tile_pool(name="tree", bufs=treep_bufs) as treep:
                    for qcg in range(0, NQC, tree_qcs):
                        simc = simsb.tile(
                            [128, tree_qcs * 4 * n_act, Ld], F16, tag="simc")
                        for qi in range(tree_qcs):
                            qc = qcg + qi
                            nd = n_dve + (1 if (alt_dve and qc % 2) else 0)
                            na = 4 - nd
                            lhs = qT[:, qc * 128:(qc + 1) * 128]
                            order = [t for t in range(4) if t != dve_pos]
                            order = ([dve_pos] + order if dve_first
                                     else order[:dve_pos] + [dve_pos]
                                     + order[dve_pos:])
                            assert not (dve_first and alt_dve)
                            for t in order:
                                sim = simps.tile([128, 4, Ld], F32, tag="sim")
                                for b in range(2):
                                    pr = 4 * t + 2 * b
                                    nc.tensor.matmul(
                                        sim[:, 2 * b:2 * b + 2, :], lhs,
                                        dT_s[:, pr:pr + 2, :], start=True,
                                        stop=True)
                                if split_tile and t == na - 1:
                                    # 3 pairs via ACT copy, 1 via direct DVE
                                    nc.scalar.copy(
                                        simc[:, 4 * t:4 * t + 3, :],
                                        sim[:, 0:3, :])
                                    nc.vector.reduce_max(
                                        maxtile[:, qc, 4 * t + 3:4 * t + 4],
                                        sim[:, 3:4, :], axis=AX)
                                elif t < na:
                                    nc.scalar.copy(
                                        simc[:, qi * 4 * n_act + 4 * t:
                                             qi * 4 * n_act + 4 * t + 4, :],
                                        sim[:])
                                else:
                                    nc.vector.reduce_max(
                                        maxtile[:, qc, 4 * t:4 * t + 4],
                                        sim[:], axis=AX)
                        np_ = 4 * na if alt_dve else tree_qcs * 4 * n_act
                        if split_tile:
                            np_ = 4 * n_act - 1
                        t1 = treep.tile([128, tree_qcs * 4 * n_act, 128], F16,
                                        tag="t1")
                        nc.vector.tensor_tensor(
                            t1[:, 0:np_, :], simc[:, 0:np_, 0:128],
                            simc[:, 0:np_, 128:256], op=MAX)
                        t2 = treep.tile([128, tree_qcs * 4 * n_act, 64], F16,
                                        tag="t2")
                        nc.vector.tensor_tensor(
                            t2[:, 0:np_, :], t1[:, 0:np_, 0:64],
                            t1[:, 0:np_, 64:128], op=MAX)
                        t3 = treep.tile([128, tree_qcs * 4 * n_act, 32], F16,
                                        tag="t3")
                        nc.vector.tensor_tensor(
                            t3[:, 0:np_, :], t2[:, 0:np_, 0:32],
                            t2[:, 0:np_, 32:64], op=MAX)
                        t4 = treep.tile([128, tree_qcs * 4 * n_act, 16], F16,
                                        tag="t4")
                        nc.vector.tensor_tensor(
                            t4[:, 0:np_, :], t3[:, 0:np_, 0:16],
                            t3[:, 0:np_, 16:32], op=MAX)
                        t5 = treep.tile([128, tree_qcs * 4 * n_act, 8], F16,
                                        tag="t5")
                        nc.vector.tensor_tensor(
                            t5[:, 0:np_, :], t4[:, 0:np_, 0:8],
                            t4[:, 0:np_, 8:16], op=MAX)
                        for qi in range(tree_qcs):
                            qc = qcg + qi
                            w = np_ if (alt_dve or split_tile) else 4 * n_act
                            nc.vector.reduce_max(
                                maxtile[:, qc, 0:w],
                                t5[:, qi * 4 * n_act:qi * 4 * n_act + w, :],
                                axis=AX)
                            s_ps = simps.tile(
                                [P_LOC, 4], F32, tag="sprs", bufs=2)
                            nc.tensor.matmul(
                                s_ps[:], maxtile[:, qc, :], ind_t[:],
                                start=True, stop=True)
                            if s_act:
                                nc.scalar.copy(
                                    S_sbuf[:, 4 * qc:4 * qc + 4], s_ps[:])
                            else:
                                nc.vector.tensor_copy(
                                    S_sbuf[:, 4 * qc:4 * qc + 4], s_ps[:])
                simps_cm.__exit__(None, None, None)

                if dma_split:
                    for g4 in range(0, NQC, 4):
                        nc.sync.dma_start(
                            out=s_out[:, 4 * g4:4 * g4 + 16],
                            in_=S_sbuf[:, 4 * g4:4 * g4 + 16])
                else:
                    nc.sync.dma_start(out=s_out[:], in_=S_sbuf[:])

    nc.finalize()
    return nc


def _get_program():
    global _PROG
    if _PROG is None:
        _PROG = _build_program()
    return _PROG


def _logsumexp(x, axis):
    m = np.max(x, axis=axis, keepdims=True)
    return (m + np.log(np.sum(np.exp(x - m), axis=axis, keepdims=True)))


def kernel(q_reps, d_reps, d_masks, labels):
    nc = _get_program()
    q_reps = np.asarray(q_reps, dtype=np.float32)
    d_reps = np.asarray(d_reps, dtype=np.float32)
    d_masks = np.asarray(d_masks)
    labels = np.asarray(labels, dtype=np.float32)

    # Host-side input layout prep (mask, shard, transpose so that the
    # contraction dim D sits on SBUF partitions).
    d = d_reps * d_masks[..., None].astype(d_reps.dtype)          # [N,B,Ld,D]
    dT = np.transpose(d.reshape(PAIRS, Ld, D), (2, 0, 1))          # [D,pair,Ld]
    q = np.ascontiguousarray(
        q_reps.reshape(QTOK, D).reshape(NQC, 128, D), dtype=np.float32)
    ind = np.zeros((128, 4), np.float16)
    ind[np.arange(128), np.arange(128) // 32] = 1
    ident = np.eye(128, dtype=np.float16)

    in_maps = []
    for c in range(N_CORES):
        in_maps.append({
            "dt": np.ascontiguousarray(
                dT[:, c * P_LOC:(c + 1) * P_LOC, :]).astype(np.float16),
            "q": q.astype(np.float16),
            "ind": ind,
            "ident": ident,
        })
    res = run_bass_kernel_spmd(nc, in_maps, list(range(N_CORES)))

    # Gather the per-core [16, 64] score slabs -> S[n, bq, bd].
    S_full = np.concatenate(
        [res.results[c]["s"] for c in range(N_CORES)], axis=0)     # [128, 64]
    S = S_full.reshape(N, B, B).transpose(0, 2, 1).astype(np.float64)

    # Tiny scalar finale (softmax losses over the gathered 128x64 slab).
    lab = labels.astype(np.float64)
    idx = np.arange(B)
    scores = S[:, idx, idx].T                                      # [B, N]
    log_p = scores - _logsumexp(scores, axis=-1)
    kl = np.sum(np.exp(lab) * (lab - log_p)) / B
    pred = S.transpose(1, 0, 2).reshape(B, N * B)                  # [B, N*B]
    ce = -np.mean(pred[idx, idx] - _logsumexp(pred, axis=-1)[:, 0])
    return np.float32(kl + INBATCH_P * ce)

